# revision 5
# baseline (speedup 1.0000x reference)
"""Self-contained Trainium2 Bass kernel for nn_ChamferVAELoss (fast path).

Approach: the scaled pairwise chamfer matrix q[np, nt] = -16*se[np, nt]
(se = mean_d smoothl1(pred_d - real_d)) is computed as a rank-8-per-
coordinate separable expansion

    q[np, nt] = sum_{k<8, d<8} psi_x[(k,d), np] * psi_y[(k,d), nt]

where psi = Phi(x) @ C with a feature dictionary Phi = [x, x^2,
tanh(s_l (x - c_l)) for 24 atoms], fitted offline (weighted ALS on the
displacement kernel -2*smoothl1(x-y)).  End-to-end sim error vs the exact
reference is ~8e-4 relative (tolerance 2e-2).

Device pipeline per core (2 batches, both sides):
  - host sends x and x^2 as bf16 [SB, 8, N] per side
  - PE replicates x across 16/8 atom groups (constant matmul -> PSUM)
  - ACT applies tanh with per-partition scale/bias -> phi tiles (bf16)
  - PE combination matmuls (C in hi/lo bf16) -> psi PSUM, ACT evac -> bf16
  - PE main matmul per 128-row block: q = psi_x(slice).T @ psi_y  (K=64)
  - DVE fused evac+rowmax: tensor_scalar(out=se bf16, op0=mult 1.0,
    accum_out=rowmax col, op1=max); folds (tensor_tensor max) build the
    per-batch column-max tile; PE transposes + small DVE reduces finish
    the column max; final adds produce res[128, 1] per batch.
  - host sums the partials: loss = -total / (16 * B * N).

The dkl term in the reference is multiplied by 0.0 and contributes
nothing for finite inputs, so mu/log_var are ignored.
"""
import numpy as np
from contextlib import ExitStack
import concourse.bass as bass
import concourse.mybir as mybir

F32 = mybir.dt.float32
BF16 = mybir.dt.bfloat16
AX = mybir.AxisListType
OP = mybir.AluOpType
AF = mybir.ActivationFunctionType

SB = 2          # batches per core
N = 1024
D = 8
R = 8           # rank (psi features per coordinate)
NTANH = 24
M = NTANH + 2   # dictionary size: x, x^2, 24 tanh atoms
NA = 16         # atoms in tile A (tanh 0..15)
NB = 8          # atoms in tile B (tanh 16..23)

import json as _json
_CONSTS = _json.loads(r"""__CONSTS_JSON__""")
CENTS = np.array(_CONSTS["CENTS"], dtype=np.float64)
SLOPES = np.array(_CONSTS["SLOPES"], dtype=np.float64)
CXB = np.array(_CONSTS["CX"], dtype=np.float64)   # [M, R] balanced
CYB = np.array(_CONSTS["CY"], dtype=np.float64)


class _FakeInst:
    def then_inc(self, *a, **k):
        return self


class _FakeEngine:
    def __getattr__(self, name):
        return lambda *a, **k: _FakeInst()


class Sched:
    def __init__(self, sems, dry, ev=None):
        self.sems = sems
        self.dry = dry
        self.count = {k: 0 for k in ["dma", "pe", "act", "dve"]}
        self.ev = {} if ev is None else ev

    def emit(self, engine_name, inst, inc, event=None):
        if not self.dry:
            inst.then_inc(self.sems[engine_name], inc)
        self.count[engine_name] += inc
        if event is not None:
            if self.dry:
                self.ev[event] = (engine_name, self.count[engine_name])
            else:
                assert self.ev[event] == (engine_name, self.count[engine_name]), event
        return inst

    def wait(self, engine, event):
        if self.dry:
            return
        src, val = self.ev[event]
        engine.wait_ge(self.sems[src], val)

    def wait_if(self, engine, event):
        if self.dry or event in self.ev:
            if event in self.ev:
                self.wait(engine, event)


def build_nc(reps=1, debug=False):
    nc = bass.Bass()
    # DRAM inputs: per-side bf16 x and x^2, layout [SB, D, N]
    xh = nc.dram_tensor("xh", [SB, D, N], BF16, kind="ExternalInput")
    x2h = nc.dram_tensor("x2h", [SB, D, N], BF16, kind="ExternalInput")
    yh = nc.dram_tensor("yh", [SB, D, N], BF16, kind="ExternalInput")
    y2h = nc.dram_tensor("y2h", [SB, D, N], BF16, kind="ExternalInput")
    # constants (separate combination matrices per side: x=pred, y=real)
    c_ct = {}
    for sd in ("x", "y"):
        c_ct[f"{sd}_ctAh"] = nc.dram_tensor(f"{sd}ctAh", [NA * D, R * D], BF16, kind="ExternalInput")
        c_ct[f"{sd}_ctAl"] = nc.dram_tensor(f"{sd}ctAl", [NA * D, R * D], BF16, kind="ExternalInput")
        c_ct[f"{sd}_ctBh"] = nc.dram_tensor(f"{sd}ctBh", [NB * D, R * D], BF16, kind="ExternalInput")
        c_ct[f"{sd}_ctBl"] = nc.dram_tensor(f"{sd}ctBl", [NB * D, R * D], BF16, kind="ExternalInput")
        c_ct[f"{sd}_cph"] = nc.dram_tensor(f"{sd}cph", [2 * D, R * D], BF16, kind="ExternalInput")
        c_ct[f"{sd}_cpl"] = nc.dram_tensor(f"{sd}cpl", [2 * D, R * D], BF16, kind="ExternalInput")
    c_repA = nc.dram_tensor("repA", [D, NA * D], BF16, kind="ExternalInput")
    c_repB = nc.dram_tensor("repB", [D, NB * D], BF16, kind="ExternalInput")
    c_sid = nc.dram_tensor("sid", [128, 128], BF16, kind="ExternalInput")
    c_scA = nc.dram_tensor("scA", [NA * D, 1], F32, kind="ExternalInput")
    c_biA = nc.dram_tensor("biA", [NA * D, 1], F32, kind="ExternalInput")
    c_scB = nc.dram_tensor("scB", [NB * D, 1], F32, kind="ExternalInput")
    c_biB = nc.dram_tensor("biB", [NB * D, 1], F32, kind="ExternalInput")
    res = [nc.dram_tensor(f"res{b}", [128, 1], F32, kind="ExternalOutput")
           for b in range(SB)]
    if debug:
        dbg_psix = nc.dram_tensor("dbg_psix", [R * D, N], F32, kind="ExternalOutput")
        dbg_psiy = nc.dram_tensor("dbg_psiy", [R * D, N], F32, kind="ExternalOutput")
        dbg_se0 = nc.dram_tensor("dbg_se0", [128, N], F32, kind="ExternalOutput")
        dbg_cmax = nc.dram_tensor("dbg_cmax", [128, N], F32, kind="ExternalOutput")
        dbg_rmaxc = nc.dram_tensor("dbg_rmaxc", [128, R], F32, kind="ExternalOutput")
        dbg_cmaxc = nc.dram_tensor("dbg_cmaxc", [128, R], F32, kind="ExternalOutput")

    es = ExitStack()
    sbuf = lambda name, shape, dt=F32: es.enter_context(nc.sbuf_tensor(name, shape, dt))

    s_ct = {}
    for sd in ("x", "y"):
        s_ct[f"{sd}_ctAh"] = sbuf(f"s_{sd}ctAh", [NA * D, R * D], BF16)
        s_ct[f"{sd}_ctAl"] = sbuf(f"s_{sd}ctAl", [NA * D, R * D], BF16)
        s_ct[f"{sd}_ctBh"] = sbuf(f"s_{sd}ctBh", [NB * D, R * D], BF16)
        s_ct[f"{sd}_ctBl"] = sbuf(f"s_{sd}ctBl", [NB * D, R * D], BF16)
        s_ct[f"{sd}_cph"] = sbuf(f"s_{sd}cph", [2 * D, R * D], BF16)
        s_ct[f"{sd}_cpl"] = sbuf(f"s_{sd}cpl", [2 * D, R * D], BF16)
    s_repA = sbuf("s_repA", [D, NA * D], BF16)
    s_repB = sbuf("s_repB", [D, NB * D], BF16)
    s_sid = sbuf("s_sid", [128, 128], BF16)
    s_scA = sbuf("s_scA", [NA * D, 1])
    s_biA = sbuf("s_biA", [NA * D, 1])
    s_scB = sbuf("s_scB", [NB * D, 1])
    s_biB = sbuf("s_biB", [NB * D, 1])

    # staged per-(side, batch) poly tiles: rows 0..7 = x, rows 8..15 = x^2
    P = [[sbuf(f"P{s}{b}", [2 * D, N], BF16) for b in range(SB)] for s in range(2)]
    phiA = [sbuf(f"phiA{s}", [NA * D, N], BF16) for s in range(2)]
    phiB = [sbuf(f"phiB{s}", [NB * D, N], BF16) for s in range(2)]
    psi = [[sbuf(f"psi{s}{b}", [R * D, N], BF16) for b in range(SB)] for s in range(2)]
    se = [sbuf(f"se{i}", [128, N], BF16) for i in range(2)]
    cmax = [sbuf(f"cmax{b}", [128, N], BF16) for b in range(SB)]
    rmaxc = [sbuf(f"rmaxc{b}", [128, 8]) for b in range(SB)]
    cmaxc = [sbuf(f"cmaxc{b}", [128, 8]) for b in range(SB)]
    ra = [sbuf(f"ra{b}", [128, 1]) for b in range(SB)]
    rc = [sbuf(f"rc{b}", [128, 1]) for b in range(SB)]
    res_b = [sbuf(f"resb{b}", [128, 1]) for b in range(SB)]
    if debug:
        d_psix = sbuf("d_psix", [R * D, N])
        d_psiy = sbuf("d_psiy", [R * D, N])
        d_se0 = sbuf("d_se0", [128, N])
        d_cmax = sbuf("d_cmax", [128, N])

    qA = es.enter_context(nc.psum_tensor("qA", [128, N], F32))
    qB = es.enter_context(nc.psum_tensor("qB", [128, N], F32))
    pRep = es.enter_context(nc.psum_tensor("pRep", [128, N], F32))
    pPsi = es.enter_context(nc.psum_tensor("pPsi", [R * D, N], F32))

    sems = {
        "dma": es.enter_context(nc.semaphore("dma_sem")),
        "pe": es.enter_context(nc.semaphore("t_sem")),
        "act": es.enter_context(nc.semaphore("a_sem")),
        "dve": es.enter_context(nc.semaphore("v_sem")),
    }

    SBORD = [(0, 0), (1, 0), (0, 1), (1, 1)]   # (side, batch) build order

    def body_sync(sync, S):
        # constants once
        for key in s_ct:
            S.emit("dma", sync.dma_start(s_ct[key][:], c_ct[key][:]), 16)
        S.emit("dma", sync.dma_start(s_repA[:], c_repA[:]), 16)
        S.emit("dma", sync.dma_start(s_repB[:], c_repB[:]), 16)
        S.emit("dma", sync.dma_start(s_sid[:], c_sid[:]), 16)
        S.emit("dma", sync.dma_start(s_scA[:], c_scA[:]), 16)
        S.emit("dma", sync.dma_start(s_biA[:], c_biA[:]), 16)
        S.emit("dma", sync.dma_start(s_scB[:], c_scB[:]), 16)
        S.emit("dma", sync.dma_start(s_biB[:], c_biB[:]), 16, "dma_consts")
        # self-wait so the DGE batch breaks at the event boundary (the race
        # detector only allows waits at batch-end counts)
        S.wait(sync, "dma_consts")
        for rr in range(reps):
            for io, (s, b) in enumerate(SBORD):
                if rr > 0:
                    S.wait(sync, f"mmPsi{s}{b}_{rr-1}")
                elif io > 0:
                    ps, pb = SBORD[io - 1]
                    S.wait(sync, f"dmaP{ps}{pb}_{0}")
                src1 = xh[b] if s == 0 else yh[b]
                src2 = x2h[b] if s == 0 else y2h[b]
                S.emit("dma", sync.dma_start(P[s][b][0:D, :], src1), 16)
                S.emit("dma", sync.dma_start(P[s][b][D:2 * D, :], src2), 16,
                       f"dmaP{s}{b}_{rr}")
            for b in range(SB):
                S.wait(sync, f"tot{b}_{rr}")
                S.emit("dma", sync.dma_start(res[b][:], res_b[b][:]), 16,
                       f"dma_out{b}_{rr}")
        if debug:
            S.wait(sync, "dbg_ready")
            S.emit("dma", sync.dma_start(dbg_psix[:], d_psix[:]), 16)
            S.emit("dma", sync.dma_start(dbg_psiy[:], d_psiy[:]), 16)
            S.emit("dma", sync.dma_start(dbg_se0[:], d_se0[:]), 16)
            S.emit("dma", sync.dma_start(dbg_cmax[:], d_cmax[:]), 16)
            S.emit("dma", sync.dma_start(dbg_rmaxc[:], rmaxc[0][:]), 16)
            S.emit("dma", sync.dma_start(dbg_cmaxc[:], cmaxc[0][:]), 16)

    def body_tensor(tensor, S):
        for rr in range(reps):
            for io, (s, b) in enumerate(SBORD):
                S.wait(tensor, f"dmaP{s}{b}_{rr}")
                if rr == 0 and io == 0:
                    S.wait(tensor, "dma_consts")
                # pRep shared serially across (s, b)
                if (rr, io) != (0, 0):
                    ps, pb = SBORD[io - 1] if io > 0 else SBORD[-1]
                    prr = rr if io > 0 else rr - 1
                    S.wait(tensor, f"actB{ps}{pb}_{prr}")
                for c in range(2):
                    cs = slice(c * 512, (c + 1) * 512)
                    last = tensor.matmul(pRep[:, cs], s_repA[:], P[s][b][0:D, cs],
                                         start=True, stop=True)
                S.emit("pe", last, 1, f"mmRepA{s}{b}_{rr}")
                S.wait(tensor, f"actA{s}{b}_{rr}")
                for c in range(2):
                    cs = slice(c * 512, (c + 1) * 512)
                    last = tensor.matmul(pRep[0:NB * D, cs], s_repB[:],
                                         P[s][b][0:D, cs], start=True, stop=True)
                S.emit("pe", last, 1, f"mmRepB{s}{b}_{rr}")
                S.wait(tensor, f"actB{s}{b}_{rr}")
                # pPsi shared serially
                if (rr, io) != (0, 0):
                    ps, pb = SBORD[io - 1] if io > 0 else SBORD[-1]
                    prr = rr if io > 0 else rr - 1
                    S.wait(tensor, f"actPsi{ps}{pb}_{prr}")
                sd = "x" if s == 0 else "y"
                for c in range(2):
                    cs = slice(c * 512, (c + 1) * 512)
                    tensor.matmul(pPsi[:, cs], s_ct[f"{sd}_ctAh"][:], phiA[s][:, cs], start=True, stop=False)
                    tensor.matmul(pPsi[:, cs], s_ct[f"{sd}_ctAl"][:], phiA[s][:, cs], start=False, stop=False)
                    tensor.matmul(pPsi[:, cs], s_ct[f"{sd}_ctBh"][:], phiB[s][:, cs], start=False, stop=False)
                    tensor.matmul(pPsi[:, cs], s_ct[f"{sd}_ctBl"][:], phiB[s][:, cs], start=False, stop=False)
                    tensor.matmul(pPsi[:, cs], s_ct[f"{sd}_cph"][:], P[s][b][:, cs], start=False, stop=False)
                    last = tensor.matmul(pPsi[:, cs], s_ct[f"{sd}_cpl"][:], P[s][b][:, cs],
                                         start=False, stop=True)
                S.emit("pe", last, 1, f"mmPsi{s}{b}_{rr}")
            def mmq(b, g):
                if g >= 2:
                    S.wait(tensor, f"evac{b}{g-2}_{rr}")
                elif g == 0:
                    if b > 0:
                        S.wait(tensor, f"evac{b-1}{6}_{rr}")
                    elif rr > 0:
                        S.wait(tensor, f"evac{SB-1}{6}_{rr-1}")
                else:  # g == 1
                    if b > 0:
                        S.wait(tensor, f"cred{b-1}{7}_{rr}")
                    elif rr > 0:
                        S.wait(tensor, f"cred{SB-1}{7}_{rr-1}")
                q = qA if g % 2 == 0 else qB
                gs = slice(g * 128, (g + 1) * 128)
                for c in range(2):
                    cs = slice(c * 512, (c + 1) * 512)
                    last = tensor.matmul(q[:, cs], psi[0][b][:, gs],
                                         psi[1][b][:, cs],
                                         start=True, stop=True)
                S.emit("pe", last, 1, f"mmQ{b}{g}_{rr}")

            def mmtr(b):
                # transposes go to qB's two bank-start offsets (bf16 cols 0 and
                # 1024) only -- non-bank-start psum transpose output faults
                S.wait(tensor, f"fold{b}{7}_{rr}")
                S.wait(tensor, f"evac{b}{7}_{rr}")
                qBb = qB.ap().bitcast(BF16)
                for j in range(8):
                    half = (j % 2) * 1024
                    if j >= 2:
                        S.wait(tensor, f"cred{b}{j-2}_{rr}")
                    js = slice(j * 128, (j + 1) * 128)
                    last = tensor.transpose(qBb[:, half:half + 128],
                                            cmax[b][:, js], s_sid[:])
                    S.emit("pe", last, 1, f"mmTr{b}{j}_{rr}")

            issued_first = set()
            for b in range(SB):
                S.wait(tensor, f"actPsi{0}{b}_{rr}")
                S.wait(tensor, f"actPsi{1}{b}_{rr}")
                for g in range((1 if b in issued_first else 0), 8):
                    mmq(b, g)
                if b + 1 < SB:
                    # issue next batch's first (qA) main before the transpose
                    # block so DVE stays fed during the colmax tail
                    S.wait(tensor, f"actPsi{0}{b+1}_{rr}")
                    S.wait(tensor, f"actPsi{1}{b+1}_{rr}")
                    mmq(b + 1, 0)
                    issued_first.add(b + 1)
                mmtr(b)

    def body_scalar(scalar, S):
        for rr in range(reps):
            for io, (s, b) in enumerate(SBORD):
                S.wait(scalar, f"mmRepA{s}{b}_{rr}")
                if b > 0:
                    S.wait(scalar, f"mmPsi{s}{b-1}_{rr}")
                elif rr > 0:
                    S.wait(scalar, f"mmPsi{s}{SB-1}_{rr-1}")
                inst = scalar.activation(phiA[s][:], pRep[:], AF.Tanh,
                                         bias=s_biA.ap(), scale=s_scA.ap())
                S.emit("act", inst, 1, f"actA{s}{b}_{rr}")
                S.wait(scalar, f"mmRepB{s}{b}_{rr}")
                inst = scalar.activation(phiB[s][:], pRep[0:NB * D, :], AF.Tanh,
                                         bias=s_biB.ap(), scale=s_scB.ap())
                S.emit("act", inst, 1, f"actB{s}{b}_{rr}")
                S.wait(scalar, f"mmPsi{s}{b}_{rr}")
                if rr > 0:
                    S.wait(scalar, f"mmQ{b}{7}_{rr-1}")
                inst = scalar.activation(psi[s][b][:], pPsi[:], AF.Copy,
                                         bias=0.0, scale=1.0)
                S.emit("act", inst, 1, f"actPsi{s}{b}_{rr}")

    def body_vector(vector, S):
        for rr in range(reps):
            # order per batch: evac0, evac1, evac2, fold1, evac3, fold2, ...
            # evac7, fold6, fold7 (keeps DVE RAW distance >= 2); batch b+1's
            # evac0 is issued before batch b's creds so DVE stays fed while
            # the PE transposes run.
            def do_evac(b, g):
                S.wait(vector, f"mmQ{b}{g}_{rr}")
                q = qA if g % 2 == 0 else qB
                tgt = cmax[b] if g == 0 else se[g % 2]
                if g == 0:
                    # self-wait evidence for same-engine WAR/WAW (runtime
                    # no-op since DVE is serial)
                    if b == 0 and rr > 0:
                        S.wait(vector, f"tot{SB-1}_{rr-1}")
                    if rr > 0:
                        S.wait(vector, f"mmTr{b}{7}_{rr-1}")
                if g == 1 and b > 0:
                    S.wait(vector, f"fold{b-1}{7}_{rr}")
                if g >= 3:
                    S.wait(vector, f"fold{b}{g-2}_{rr}")
                inst = vector.tensor_scalar(tgt[:], q[:], 1.0, -3.0e38,
                                            OP.mult, OP.max,
                                            accum_out=rmaxc[b][:, g:g + 1])
                S.emit("dve", inst, 1, f"evac{b}{g}_{rr}")

            def do_fold(b, g):
                if g == 1:
                    S.wait(vector, f"evac{b}{1}_{rr}")
                else:
                    S.wait(vector, f"fold{b}{g-1}_{rr}")
                inst = vector.tensor_tensor(cmax[b][:], cmax[b][:],
                                            se[g % 2][:], OP.max)
                S.emit("dve", inst, 1, f"fold{b}{g}_{rr}")

            def do_creds(b):
                qBb = qB.ap().bitcast(BF16)
                for j in range(8):
                    half = (j % 2) * 1024
                    S.wait(vector, f"mmTr{b}{j}_{rr}")
                    inst = vector.tensor_reduce(cmaxc[b][:, j:j + 1],
                                                qBb[:, half:half + 128],
                                                AX.X, OP.max)
                    S.emit("dve", inst, 1, f"cred{b}{j}_{rr}")

            for b in range(SB):
                start_g = 1 if b > 0 else 0
                for g in range(start_g, 8):
                    do_evac(b, g)
                    if g >= 2:
                        do_fold(b, g - 1)
                do_fold(b, 7)
                if b + 1 < SB:
                    do_evac(b + 1, 0)
                do_creds(b)
            if debug and rr == reps - 1:
                S.emit("dve", vector.tensor_copy(d_se0[:], se[1][:]), 1)
                S.emit("dve", vector.tensor_copy(d_cmax[:], cmax[0][:]), 1)
                S.emit("dve", vector.tensor_copy(d_psix[:], psi[0][0][:]), 1)
                S.emit("dve", vector.tensor_copy(d_psiy[:], psi[1][0][:]), 1,
                       "dbg_ready")
            for b in range(SB):
                S.wait(vector, f"cred{b}{7}_{rr}")
                S.emit("dve", vector.tensor_reduce(ra[b][:], rmaxc[b][:],
                                                   AX.X, OP.add), 1, f"rsum{b}_{rr}")
                S.emit("dve", vector.tensor_reduce(rc[b][:], cmaxc[b][:],
                                                   AX.X, OP.add), 1, f"csum{b}_{rr}")
            for b in range(SB):
                if rr > 0:
                    S.wait(vector, f"dma_out{b}_{rr-1}")
                S.wait(vector, f"csum{b}_{rr}")
                S.emit("dve", vector.tensor_tensor(res_b[b][:], ra[b][:], rc[b][:],
                                                   OP.add), 1, f"tot{b}_{rr}")

    S0 = Sched(sems, dry=True)
    fake = _FakeEngine()
    body_sync(fake, S0)
    body_tensor(fake, S0)
    body_scalar(fake, S0)
    body_vector(fake, S0)

    S1 = Sched(sems, dry=False, ev=S0.ev)
    block = es.enter_context(nc.Block())

    @block.sync
    def _(sync):
        body_sync(sync, S1)

    @block.tensor
    def _(tensor):
        body_tensor(tensor, S1)

    @block.scalar
    def _(scalar):
        body_scalar(scalar, S1)

    @block.vector
    def _(vector):
        body_vector(vector, S1)

    es.close()
    return nc


# ---------------------------------------------------------------------------
# host-side constants
# ---------------------------------------------------------------------------
def _make_host_consts():
    import ml_dtypes
    bf = ml_dtypes.bfloat16

    def hl(a):
        h = a.astype(bf)
        l = (a - h.astype(np.float64)).astype(bf)
        return h, l

    # combination constants: ct[(l, d), (k, d')] = C[row_l, k] * delta(d, d')
    def expand(Crows):
        nl = Crows.shape[0]
        out = np.zeros((nl * D, R * D), dtype=np.float64)
        for l in range(nl):
            for k in range(R):
                for d in range(D):
                    out[l * D + d, k * D + d] = Crows[l, k]
        return out

    consts = {}
    for (tag, C) in (("x", CXB), ("y", CYB)):
        ctA = expand(C[2:2 + NA])        # tanh atoms 0..15
        ctB = expand(C[2 + NA:2 + NA + NB])
        cp = expand(C[0:2])              # x, x^2 rows
        for nm, arr in (("ctA", ctA), ("ctB", ctB), ("cp", cp)):
            h, l = hl(arr)
            consts[f"{tag}_{nm}h"] = h
            consts[f"{tag}_{nm}l"] = l
    consts["repA"] = np.tile(np.eye(D), (1, NA)).astype(bf)
    consts["repB"] = np.tile(np.eye(D), (1, NB)).astype(bf)
    consts["sid"] = np.eye(128).astype(bf)
    scA = np.zeros((NA * D, 1), np.float32)
    biA = np.zeros((NA * D, 1), np.float32)
    for l in range(NA):
        scA[l * D:(l + 1) * D, 0] = SLOPES[l]
        biA[l * D:(l + 1) * D, 0] = -SLOPES[l] * CENTS[l]
    scB = np.zeros((NB * D, 1), np.float32)
    biB = np.zeros((NB * D, 1), np.float32)
    for i, l in enumerate(range(NA, NA + NB)):
        scB[i * D:(i + 1) * D, 0] = SLOPES[l]
        biB[i * D:(i + 1) * D, 0] = -SLOPES[l] * CENTS[l]
    consts["scA"], consts["biA"] = scA, biA
    consts["scB"], consts["biB"] = scB, biB
    return consts


_HOST_CONSTS = None


def host_consts():
    global _HOST_CONSTS
    if _HOST_CONSTS is None:
        _HOST_CONSTS = _make_host_consts()
    return _HOST_CONSTS


# ---------------------------------------------------------------------------
# PJRT runner (built once, cached)
# ---------------------------------------------------------------------------
import jax
from jax.experimental.shard_map import shard_map
from jax.sharding import Mesh, PartitionSpec
from concourse.bass2jax import _bass_exec_p, partition_id_tensor, install_neuronx_cc_hook


def make_runner(nc, n_cores):
    install_neuronx_cc_hook()
    partition_name = nc.partition_id_tensor.name if nc.partition_id_tensor else None

    in_names, out_names, out_avals, zero_outs = [], [], [], []
    for alloc in nc.m.functions[0].allocations:
        if not isinstance(alloc, mybir.MemoryLocationSet):
            continue
        name = alloc.memorylocations[0].name
        if alloc.kind == "ExternalInput":
            if name != partition_name:
                in_names.append(name)
        elif alloc.kind == "ExternalOutput":
            out_names.append(name)
            shape = tuple(alloc.tensor_shape)
            dtype = mybir.dt.np(alloc.dtype)
            out_avals.append(jax.core.ShapedArray(shape, dtype))
            zero_outs.append(np.zeros(shape, dtype))
    n_params = len(in_names)
    n_outs = len(out_avals)
    all_in_names = list(in_names) + list(out_names)
    if partition_name is not None:
        all_in_names.append(partition_name)
    donate = tuple(range(n_params, n_params + n_outs))

    def _body(*args):
        operands = list(args)
        if partition_name is not None:
            operands.append(partition_id_tensor())
        outs = _bass_exec_p.bind(
            *operands,
            out_avals=tuple(out_avals),
            in_names=tuple(all_in_names),
            out_names=tuple(out_names),
            lowering_input_output_aliases=(),
            sim_require_finite=False,
            sim_require_nnan=False,
            nc=nc,
        )
        return tuple(outs)

    devices = jax.devices()[:n_cores]
    mesh = Mesh(np.asarray(devices), ("core",))
    in_specs = (PartitionSpec("core"),) * (n_params + n_outs)
    out_specs = (PartitionSpec("core"),) * n_outs
    fn = jax.jit(
        shard_map(_body, mesh=mesh, in_specs=in_specs, out_specs=out_specs,
                  check_rep=False),
        donate_argnums=donate, keep_unused=True,
    )

    def run(in_maps):
        global_ins = [
            np.concatenate([np.asarray(m[name]) for m in in_maps], axis=0)
            for name in in_names
        ]
        global_zeros = [
            np.concatenate([z] * n_cores, axis=0) for z in zero_outs
        ]
        out_arrs = fn(*global_ins, *global_zeros)
        out_splits = [np.split(np.asarray(a), n_cores, axis=0) for a in out_arrs]
        return [
            {name: out_splits[i][c] for i, name in enumerate(out_names)}
            for c in range(n_cores)
        ]

    run.out_names = out_names
    run.in_names = in_names
    return run


_CACHE = {}


def _get_runner(reps=1, debug=False):
    key = (reps, debug)
    if key not in _CACHE:
        nc = build_nc(reps=reps, debug=debug)
        _CACHE[key] = make_runner(nc, 8)
    return _CACHE[key]


def make_in_maps(output, real):
    import ml_dtypes
    bf = ml_dtypes.bfloat16
    NC = 8
    output = np.asarray(output, dtype=np.float32)
    real = np.asarray(real, dtype=np.float32)
    cst = host_consts()
    in_maps = []
    for c in range(NC):
        sl = slice(c * SB, (c + 1) * SB)
        # [SB, N, D] -> [SB, D, N]
        x = np.ascontiguousarray(output[sl].transpose(0, 2, 1))
        y = np.ascontiguousarray(real[sl].transpose(0, 2, 1))
        m = {
            "xh": x.astype(bf), "x2h": (x * x).astype(bf),
            "yh": y.astype(bf), "y2h": (y * y).astype(bf),
            "repA": cst["repA"], "repB": cst["repB"], "sid": cst["sid"],
            "scA": cst["scA"], "biA": cst["biA"],
            "scB": cst["scB"], "biB": cst["biB"],
        }
        for sd in ("x", "y"):
            for nm in ("ctAh", "ctAl", "ctBh", "ctBl", "cph", "cpl"):
                m[f"{sd}{nm}"] = cst[f"{sd}_{nm}"]
        in_maps.append(m)
    return in_maps


def kernel(output, mu, log_var, real):
    B = 16
    in_maps = make_in_maps(output, real)
    outs = _get_runner()(in_maps)
    total = 0.0
    for c in range(8):
        for b in range(SB):
            total += float(outs[c][f"res{b}"].sum())
    ch = -total / (16.0 * B * N)
    return np.array([ch], dtype=np.float32)
